# revision 24
# baseline (speedup 1.0000x reference)
"""GrokkingSNN forward on 8 TRN2 NeuronCores.

Math notes
----------
Reference loop (thr=1 after clamps, beta1=beta2=beta):
    m_t = beta*m_{t-1} + c - s_{t-1},  s_t = H(m_t - 1),  m_0 = 0, s_0 = 0
    out = W2 @ S + G*b2,  S = sum_t beta^(15-t) s_t,  G = (1-beta^15)/(1-beta)

With z = m - m*, m* = c/(1-beta), q = 1 - m* (so s_t = [z_t > q]) the
spike recurrence telescopes against the output weights:
    S = beta^16*q - z16 - beta^16,   z16 = beta*z15 - s15
so the device only needs z16.  The -beta^16 constant folds into b2.

x has only 97*97 = 9409 distinct rows, so the network is evaluated once
per distinct (x0, x1) pair on a fixed grid; the final answer is a row
gather by pair id.  cur1 for the grid is built on-device from one-hot
matmuls (the one-hots are generated on device from the v0/v1 index rows)
against the embedding-folded weights A1/A2, split hi/lo in bf16 so the
PE gather stays exact to ~2^-17 while running at bf16 rate.

The 15-step spike recurrence runs as SIX fused custom-DVE ops per
hidden tile (46 ALU stages total) on the premultiplied state w = beta*z
and threshold qb = beta*q:
    OPEN(qb)   -> w3    (s1 via a constant compare, then 2 full steps)
    WGZ(w,qb)  -> z+3   (3 spike decisions, 8 ALUs)
    ZGW(z,qb)  -> w+2   (2 spike decisions, 7 ALUs)
chained OPEN,WGZ,ZGW,WGZ,ZGW,WGZ = z16.  S~ = beta^15*qb - z16 via one
scalar_tensor_tensor.  qb itself is produced on the Scalar engine
(activation Identity with per-partition bias) straight from PSUM.
"""

import os
import sys

import numpy as np

for _p in ("/opt/trn_rl_repo",):
    if _p not in sys.path and os.path.isdir(_p):
        sys.path.insert(0, _p)

P = 97          # vocab / output dim
H = 512         # hidden
NSTEPS = 15
NCORES = 8
NGRID = P * P   # 9409 distinct input pairs
NPAD = 9472     # 8 * 1184
NLOC = NPAD // NCORES   # 1184 grid columns per core
HT = H // 128   # 4 hidden tiles
CHUNKS = [(0, 512), (512, 512), (1024, NLOC - 1024)]  # psum-sized col chunks

_CACHE = {}
_DVE_OPS = {}


def _register_dve_ops():
    """Register the three fused recurrence ops (documented extension
    path: define a DveOp and append to dve_ops.OPS)."""
    if _DVE_OPS:
        return _DVE_OPS
    import concourse.dve_ops as dvo
    from concourse.dve_spec import (Spec, Src0, Src1, C0, C1, C2, Idx, Zero,
                                    lower, _has_src1)
    from concourse.dve_uop import DveOpSpec

    def f32(a):
        return np.asarray(a, np.float32)

    # NIOTA: out[p, k] = -k  (negated element index; Src0 ignored)
    NIOTA_spec = Spec(
        body=(Zero - Idx) + Src0 * C0,
        reference=lambda in0, in1, s0, s1, imm2: (
            -np.broadcast_to(np.arange(in0.shape[-1], dtype=np.float32),
                             in0.shape) + in0 * s0).astype(np.float32),
    )

    # OPEN: in0 = qb (= beta*q).  s1 = [qb < c0]; z2 = qb*c1 - c2 - s1;
    # w2 = z2*c1; s2 = [w2 > qb]; z3 = w2 - s2; out = z3*c1   (c1 = beta)
    _s1 = Src0 < C0
    _z2 = (Src0 * C1 - C2) - _s1
    _w2 = _z2 * C1
    _z3 = _w2 - (_w2 > Src0)
    OPEN_spec = Spec(
        body=_z3 * C1,
        reference=lambda in0, in1, s0, s1, imm2: (lambda qb, s1v: (
            lambda z2: (lambda w2: (lambda z3: f32(z3 * s1))(
                f32(w2 - (w2 > qb))))(f32(z2 * s1)))(
            f32(f32(f32(qb * s1) - imm2) - s1v)))(
            f32(in0), f32(in0 < s0)),
    )

    # WGZ: in0 = w_t, in1 = qb: three spike decisions -> z_{t+3}
    _a2 = Src0 - (Src0 > Src1)
    _a3 = _a2 * C0
    _a5 = _a3 - (_a3 > Src1)
    _a6 = _a5 * C0
    WGZ_spec = Spec(
        body=_a6 - (_a6 > Src1),
        reference=lambda in0, in1, s0, s1, imm2: (lambda w, qb, b: (
            lambda z1: (lambda w1: (lambda z2: (lambda w2: f32(w2 - (w2 > qb)))(
                f32(z2 * b)))(f32(w1 - (w1 > qb))))(f32(z1 * b)))(
            f32(w - (w > qb))))(f32(in0), f32(in1), s0),
    )

    # ZGW: in0 = z_t, in1 = qb: two spike decisions -> w_{t+2}
    _b1 = Src0 * C0
    _b3 = _b1 - (_b1 > Src1)
    _b4 = _b3 * C0
    _b6 = _b4 - (_b4 > Src1)
    ZGW_spec = Spec(
        body=_b6 * C0,
        reference=lambda in0, in1, s0, s1, imm2: (lambda z, qb, b: (
            lambda w: (lambda z1: (lambda w1: (lambda z2: f32(z2 * b))(
                f32(w1 - (w1 > qb))))(f32(z1 * b)))(f32(w - (w > qb))))(
            f32(z * b)))(f32(in0), f32(in1), s0),
    )

    for name, spec in (("SNN_OPEN", OPEN_spec), ("SNN_WGZ", WGZ_spec),
                       ("SNN_ZGW", ZGW_spec), ("SNN_NIOTA", NIOTA_spec)):
        if name not in dvo._SUB_OPCODE_FOR_NAME:
            row = dvo._CUSTOM_DVE_ROW_BASE + len(dvo.OPS)
            assert row < 0x20
            dvo._SUB_OPCODE_FOR_NAME[name] = row
            shas = {}
            for ver in ("v3", "v4"):
                uops = lower(spec, ver=ver)
                shas[ver] = DveOpSpec(name=name, opcode=row, uops=uops,
                                      rd1_en=_has_src1(spec)).sha(ver)
            # OPEN is single-stream -> eligible for the 2x_2P fp32 perf mode
            perf = {"v3": True, "v4": True} if name == "SNN_OPEN" else {}
            op = dvo.DveOp(name, spec, subdim=False, uops_sha=shas,
                           perf_en=perf)
            dvo.OPS.append(op)
            dvo.CUSTOM_DVE_SPECS[name] = spec
        _DVE_OPS[name] = next(o for o in dvo.OPS if o.name == name)
    return _DVE_OPS


def _build_bass(beta: float, thr: float):
    from concourse import bacc, mybir
    from concourse.tile import TileContext

    ops = _register_dve_ops()
    f32 = mybir.dt.float32
    bf16 = mybir.dt.bfloat16
    f16 = mybir.dt.float16
    Alu = mybir.AluOpType
    Act = mybir.ActivationFunctionType

    nc = bacc.Bacc("TRN2", target_bir_lowering=False, debug=False,
                   num_devices=NCORES)

    # A1P/A2P: [97, HT*256] — per h-tile block of (hi 128 | lo 128) bf16 cols
    dVR = nc.dram_tensor("VR", (1, 2 * NLOC), bf16, kind="ExternalInput")
    dA1 = nc.dram_tensor("A1P", (P, 2 * H), bf16, kind="ExternalInput")
    dA2 = nc.dram_tensor("A2P", (P, 2 * H), bf16, kind="ExternalInput")
    dW2 = nc.dram_tensor("W2TP", (128, HT * P), f16, kind="ExternalInput")
    dQB = nc.dram_tensor("QBB", (128, HT), f32, kind="ExternalInput")
    dB2E = nc.dram_tensor("B2E", (P, 1), f32, kind="ExternalInput")
    dOUT = nc.dram_tensor("out", (P, NLOC), f16, kind="ExternalOutput")

    b = float(beta)
    qb_scale = float(np.float32(-b / ((1.0 - b) * thr)))   # beta*kq
    c_open = float(np.float32(-b * b / (1.0 - b)))
    b2c = float(np.float32(b * b))
    b15 = float(np.float32(b ** 15))

    with TileContext(nc) as tc:
        with tc.tile_pool(name="const", bufs=1) as cpool, \
             tc.tile_pool(name="work", bufs=2) as wpool, \
             tc.tile_pool(name="psA", bufs=3, space="PSUM") as psa_pool, \
             tc.tile_pool(name="psO", bufs=1, space="PSUM") as pso_pool:

            # ---- constants into SBUF (spread across DMA rings) ----
            tVR = cpool.tile([1, 2 * NLOC], bf16, tag="tVR")
            tA1 = cpool.tile([P, 2 * H], bf16, tag="tA1")
            tA2 = cpool.tile([P, 2 * H], bf16, tag="tA2")
            tW2 = cpool.tile([128, HT * P], f16, tag="tW2")
            tQB = cpool.tile([128, HT], f32, tag="tQB")
            tB2E = cpool.tile([P, 1], f32, tag="tB2E")
            # per-tile A slices alternate the two HWDGE rings, tile 0 first
            # (DMA completion semaphores lag data by ~10us; order matters)
            nc.sync.dma_start(out=tA1[:, 0:256], in_=dA1.ap()[:, 0:256])
            nc.scalar.dma_start(out=tA2[:, 0:256], in_=dA2.ap()[:, 0:256])
            nc.sync.dma_start(out=tVR, in_=dVR.ap())
            for i in range(1, HT):
                sl = slice(i * 256, (i + 1) * 256)
                nc.sync.dma_start(out=tA1[:, sl], in_=dA1.ap()[:, sl])
                nc.scalar.dma_start(out=tA2[:, sl], in_=dA2.ap()[:, sl])
            nc.sync.dma_start(out=tQB, in_=dQB.ap())
            nc.scalar.dma_start(out=tW2, in_=dW2.ap())
            nc.scalar.dma_start(out=tB2E, in_=dB2E.ap())

            tONE = cpool.tile([1, P], bf16, tag="tONE")
            tONL = cpool.tile([1, NLOC], bf16, tag="tONL")
            tNIO = cpool.tile([1, P], bf16, tag="tNIO")
            nc.vector.memset(tONE, 1.0)
            nc.vector.memset(tONL, 1.0)
            # negated partition-index row on device (no DMA, no receipt wait)
            nc.vector._custom_dve(ops["SNN_NIOTA"], out=tNIO, in0=tONE, s0=0.0)

            # ---- one-hot O1/O2 generated on device (bf16) ----
            # ps[p, k] = v0[k] - p  via two rank-1 matmuls, then is_eq 0
            tO1 = cpool.tile([P, NLOC], bf16, tag="tO1")
            tO2 = cpool.tile([P, NLOC], bf16, tag="tO2")
            for (c0, cw) in CHUNKS:
                for r, tO in ((0, tO1), (1, tO2)):
                    ps = psa_pool.tile([128, 512], f32, tag="psA")
                    nc.tensor.matmul(ps[0:P, 0:cw], tONE,
                                     tVR[0:1, r * NLOC + c0:r * NLOC + c0 + cw],
                                     start=True, stop=False)
                    nc.tensor.matmul(ps[0:P, 0:cw], tNIO,
                                     tONL[0:1, c0:c0 + cw],
                                     start=False, stop=True)
                    nc.vector.tensor_scalar(tO[:, c0:c0 + cw], ps[0:P, 0:cw],
                                            0.0, None,
                                            Alu.is_equal, Alu.bypass)

            # persistent output psum banks, accumulated across hidden tiles
            pso = [pso_pool.tile([P, cw], f32, tag=f"pso{n}", name=f"pso{n}")
                   for n, (c0, cw) in enumerate(CHUNKS)]

            for i in range(HT):
                hi = slice(i * 256, i * 256 + 128)
                lo = slice(i * 256 + 128, (i + 1) * 256)

                # ---- stage A: cur1 psum (bf16 hi/lo one-hot gather) ----
                qb = wpool.tile([128, NLOC], f32, tag="qb")
                for (c0, cw) in CHUNKS:
                    ps = psa_pool.tile([128, 512], f32, tag="psA")
                    nc.tensor.matmul(ps[:, 0:cw], tA1[:, hi],
                                     tO1[:, c0:c0 + cw], start=True, stop=False)
                    nc.tensor.matmul(ps[:, 0:cw], tA1[:, lo],
                                     tO1[:, c0:c0 + cw], start=False, stop=False)
                    nc.tensor.matmul(ps[:, 0:cw], tA2[:, hi],
                                     tO2[:, c0:c0 + cw], start=False, stop=False)
                    nc.tensor.matmul(ps[:, 0:cw], tA2[:, lo],
                                     tO2[:, c0:c0 + cw], start=False, stop=True)
                    # qb = beta*kq*cur1_nob1 + beta*(1 + kq*b1)  on ScalarE
                    nc.scalar.activation(qb[:, c0:c0 + cw], ps[:, 0:cw],
                                         Act.Identity, bias=tQB[:, i:i + 1],
                                         scale=qb_scale)

                # ---- stage B: 15-step spike recurrence, 6 fused DVE ops ----
                st = wpool.tile([128, NLOC], f32, tag="st")
                nc.vector._custom_dve(ops["SNN_OPEN"], out=st, in0=qb,
                                      s0=c_open, s1=b, imm2=b2c)
                nc.vector._custom_dve(ops["SNN_WGZ"], out=st, in0=st, in1=qb, s0=b)
                nc.vector._custom_dve(ops["SNN_ZGW"], out=st, in0=st, in1=qb, s0=b)
                nc.vector._custom_dve(ops["SNN_WGZ"], out=st, in0=st, in1=qb, s0=b)
                nc.vector._custom_dve(ops["SNN_ZGW"], out=st, in0=st, in1=qb, s0=b)
                nc.vector._custom_dve(ops["SNN_WGZ"], out=st, in0=st, in1=qb, s0=b)

                # ---- stage C: S~ = beta^15*qb - z16 (fp16 for stage D) ----
                u = wpool.tile([128, NLOC], f16, tag="u")
                nc.vector.scalar_tensor_tensor(u, qb, b15, st,
                                               Alu.mult, Alu.subtract)

                # ---- stage D: out psum += W2T_i.T @ S_i ----
                for n, (c0, cw) in enumerate(CHUNKS):
                    nc.tensor.matmul(pso[n], tW2[:, i * P:(i + 1) * P],
                                     u[:, c0:c0 + cw],
                                     start=(i == 0), stop=(i == HT - 1))

            # ---- epilogue: add b2_eff on ScalarE, store (fp16, both rings) ----
            for n, (c0, cw) in enumerate(CHUNKS):
                ob = wpool.tile([P, cw], f16, tag=f"ob{n}")
                nc.scalar.activation(ob, pso[n], Act.Identity,
                                     bias=tB2E[:, 0:1], scale=1.0)
                eng = nc.sync if n % 2 == 0 else nc.scalar
                eng.dma_start(out=dOUT.ap()[:, c0:c0 + cw], in_=ob)

    if not nc.is_finalized():
        nc.finalize()
    return nc


def _to_bf16(a):
    import ml_dtypes
    return np.asarray(a, np.float32).astype(ml_dtypes.bfloat16)


def _prep_inputs(x, embed_w, W1, b1, W2, b2, beta, thr):
    E = embed_w.astype(np.float64)
    W1d = W1.astype(np.float64)
    A1T = np.ascontiguousarray(E @ W1d[:, :H].T).astype(np.float32)   # [97, 512]
    A2T = np.ascontiguousarray(E @ W1d[:, H:].T).astype(np.float32)

    def hi_lo_pack(A):
        # [97, HT*256]: per h-tile block of (hi 128 | lo 128) columns
        hi = _to_bf16(A).reshape(P, HT, 128)
        lo = _to_bf16(A - hi.astype(np.float32).reshape(P, H)).reshape(P, HT, 128)
        return np.ascontiguousarray(
            np.concatenate([hi, lo], axis=2).reshape(P, 2 * H))

    A1P = hi_lo_pack(A1T)   # [97, 1024] bf16
    A2P = hi_lo_pack(A2T)

    pid = np.arange(NPAD)
    v0 = (pid // P).astype(np.float32)
    v1 = (pid % P).astype(np.float32)

    kq = -1.0 / ((1.0 - beta) * thr)
    QBB = np.ascontiguousarray(
        (beta * (1.0 + kq * b1.astype(np.float64))).astype(np.float32)
        .reshape(HT, 128).T)

    G = (1.0 - beta ** NSTEPS) / (1.0 - beta)
    b2e = (G * b2.astype(np.float64)
           - (beta ** 16) * W2.astype(np.float64).sum(axis=1))
    B2E = np.ascontiguousarray(b2e.astype(np.float32).reshape(P, 1))

    # [128, 4*97] fp16: h-tile i's W2 block in cols [i*97, (i+1)*97)
    W2TP = np.ascontiguousarray(
        W2.T.astype(np.float16).reshape(HT, 128, P).transpose(1, 0, 2)
        .reshape(128, HT * P))

    in_maps = []
    for k in range(NCORES):
        sl = slice(k * NLOC, (k + 1) * NLOC)
        VR = _to_bf16(np.concatenate([v0[sl], v1[sl]]).reshape(1, 2 * NLOC))
        in_maps.append({
            "VR": np.ascontiguousarray(VR),
            "A1P": A1P, "A2P": A2P, "W2TP": W2TP, "QBB": QBB, "B2E": B2E,
        })
    return in_maps


def kernel(x, embed_w, W1, b1, W2, b2, beta1, beta2, thr1, thr2, **_):
    from concourse.bass_utils import run_bass_kernel_spmd

    beta = float(np.clip(np.float32(beta1), 0.1, 0.9))
    beta2c = float(np.clip(np.float32(beta2), 0.1, 0.9))
    thr = float(max(np.float32(thr1), 0.1))
    assert abs(beta - beta2c) < 1e-12, "kernel assumes beta1 == beta2"

    key = (round(beta, 9), round(thr, 9))
    if key not in _CACHE:
        _CACHE[key] = _build_bass(beta, thr)
    nc = _CACHE[key]

    in_maps = _prep_inputs(x, embed_w, W1, b1, W2, b2, beta, thr)
    res = run_bass_kernel_spmd(nc, in_maps, core_ids=list(range(NCORES)))
    T = np.concatenate([r["out"].astype(np.float32) for r in res.results],
                       axis=1)[:, :NGRID]

    pid = x[:, 0].astype(np.int64) * P + x[:, 1].astype(np.int64)
    return np.ascontiguousarray(T.T[pid]).astype(np.float32)


# revision 31
# speedup vs baseline: 1.1298x; 1.1298x over previous
"""GrokkingSNN forward on 8 TRN2 NeuronCores.

Math notes
----------
Reference loop (thr=1 after clamps, beta1=beta2=beta):
    m_t = beta*m_{t-1} + c - s_{t-1},  s_t = H(m_t - 1),  m_0 = 0, s_0 = 0
    out = W2 @ S + G*b2,  S = sum_t beta^(15-t) s_t,  G = (1-beta^15)/(1-beta)

With z = m - m*, m* = c/(1-beta), q = 1 - m* (so s_t = [z_t > q]) the
spike recurrence telescopes against the output weights:
    S = beta^16*q - z16 - beta^16,   z16 = beta*z15 - s15
so the device only needs z16.  The -beta^16 constant folds into b2.

x has only 97*97 = 9409 distinct rows, so the network is evaluated once
per distinct (x0, x1) pair on a fixed grid; the final answer is a row
gather by pair id.  cur1 for the grid is built on-device from one-hot
matmuls (the one-hots are generated on device from the v0/v1 index rows)
against the embedding-folded weights A1/A2, split hi/lo in bf16 so the
PE gather stays exact to ~2^-17 while running at bf16 rate.

The 15-step spike recurrence runs as SIX fused custom-DVE ops per
hidden tile (46 ALU stages total) on the premultiplied state w = beta*z
and threshold qb = beta*q:
    OPEN(qb)   -> w3    (s1 via a constant compare, then 2 full steps)
    WGZ(w,qb)  -> z+3   (3 spike decisions, 8 ALUs)
    ZGW(z,qb)  -> w+2   (2 spike decisions, 7 ALUs)
chained OPEN,WGZ,ZGW,WGZ,ZGW,WGZ = z16.  S~ = beta^15*qb - z16 via one
scalar_tensor_tensor.  qb itself is produced on the Scalar engine
(activation Identity with per-partition bias) straight from PSUM.
"""

import os
import sys

import numpy as np

for _p in ("/opt/trn_rl_repo",):
    if _p not in sys.path and os.path.isdir(_p):
        sys.path.insert(0, _p)

P = 97          # vocab / output dim
H = 512         # hidden
NSTEPS = 15
NCORES = 8
NGRID = P * P   # 9409 distinct input pairs
NPAD = 9472     # 8 * 1184
NLOC = NPAD // NCORES   # 1184 grid columns per core
HT = H // 128   # 4 hidden tiles
CHUNKS = [(0, 512), (512, 512), (1024, NLOC - 1024)]  # psum-sized col chunks

_CACHE = {}
_DVE_OPS = {}


def _register_dve_ops():
    """Register the three fused recurrence ops (documented extension
    path: define a DveOp and append to dve_ops.OPS)."""
    if _DVE_OPS:
        return _DVE_OPS
    import concourse.dve_ops as dvo
    from concourse.dve_spec import (Spec, Src0, Src1, C0, C1, C2, Idx, Zero,
                                    lower, _has_src1)
    from concourse.dve_uop import DveOpSpec

    def f32(a):
        return np.asarray(a, np.float32)

    # NIOTA: out[p, k] = -k  (negated element index; Src0 ignored)
    NIOTA_spec = Spec(
        body=(Zero - Idx) + Src0 * C0,
        reference=lambda in0, in1, s0, s1, imm2: (
            -np.broadcast_to(np.arange(in0.shape[-1], dtype=np.float32),
                             in0.shape) + in0 * s0).astype(np.float32),
    )

    # OPEN: in0 = qb (= beta*q).  s1 = [qb < c0]; z2 = qb*c1 - c2 - s1;
    # w2 = z2*c1; s2 = [w2 > qb]; z3 = w2 - s2; out = z3*c1   (c1 = beta)
    _s1 = Src0 < C0
    _z2 = (Src0 * C1 - C2) - _s1
    _w2 = _z2 * C1
    _z3 = _w2 - (_w2 > Src0)
    OPEN_spec = Spec(
        body=_z3 * C1,
        reference=lambda in0, in1, s0, s1, imm2: (lambda qb, s1v: (
            lambda z2: (lambda w2: (lambda z3: f32(z3 * s1))(
                f32(w2 - (w2 > qb))))(f32(z2 * s1)))(
            f32(f32(f32(qb * s1) - imm2) - s1v)))(
            f32(in0), f32(in0 < s0)),
    )

    # WGZ: in0 = w_t, in1 = qb: three spike decisions -> z_{t+3}
    _a2 = Src0 - (Src0 > Src1)
    _a3 = _a2 * C0
    _a5 = _a3 - (_a3 > Src1)
    _a6 = _a5 * C0
    WGZ_spec = Spec(
        body=_a6 - (_a6 > Src1),
        reference=lambda in0, in1, s0, s1, imm2: (lambda w, qb, b: (
            lambda z1: (lambda w1: (lambda z2: (lambda w2: f32(w2 - (w2 > qb)))(
                f32(z2 * b)))(f32(w1 - (w1 > qb))))(f32(z1 * b)))(
            f32(w - (w > qb))))(f32(in0), f32(in1), s0),
    )

    # ZGW: in0 = z_t, in1 = qb: two spike decisions -> w_{t+2}
    _b1 = Src0 * C0
    _b3 = _b1 - (_b1 > Src1)
    _b4 = _b3 * C0
    _b6 = _b4 - (_b4 > Src1)
    ZGW_spec = Spec(
        body=_b6 * C0,
        reference=lambda in0, in1, s0, s1, imm2: (lambda z, qb, b: (
            lambda w: (lambda z1: (lambda w1: (lambda z2: f32(z2 * b))(
                f32(w1 - (w1 > qb))))(f32(z1 * b)))(f32(w - (w > qb))))(
            f32(z * b)))(f32(in0), f32(in1), s0),
    )

    for name, spec in (("SNN_OPEN", OPEN_spec), ("SNN_WGZ", WGZ_spec),
                       ("SNN_ZGW", ZGW_spec), ("SNN_NIOTA", NIOTA_spec)):
        if name not in dvo._SUB_OPCODE_FOR_NAME:
            row = dvo._CUSTOM_DVE_ROW_BASE + len(dvo.OPS)
            assert row < 0x20
            dvo._SUB_OPCODE_FOR_NAME[name] = row
            shas = {}
            for ver in ("v3", "v4"):
                uops = lower(spec, ver=ver)
                shas[ver] = DveOpSpec(name=name, opcode=row, uops=uops,
                                      rd1_en=_has_src1(spec)).sha(ver)
            # OPEN is single-stream -> eligible for the 2x_2P fp32 perf mode
            perf = {"v3": True, "v4": True} if name == "SNN_OPEN" else {}
            op = dvo.DveOp(name, spec, subdim=False, uops_sha=shas,
                           perf_en=perf)
            dvo.OPS.append(op)
            dvo.CUSTOM_DVE_SPECS[name] = spec
        _DVE_OPS[name] = next(o for o in dvo.OPS if o.name == name)
    return _DVE_OPS


def _build_bass(beta: float, thr: float):
    from concourse import bacc, mybir
    from concourse.tile import TileContext

    ops = _register_dve_ops()
    f32 = mybir.dt.float32
    bf16 = mybir.dt.bfloat16
    f16 = mybir.dt.float16
    Alu = mybir.AluOpType
    Act = mybir.ActivationFunctionType

    nc = bacc.Bacc("TRN2", target_bir_lowering=False, debug=False,
                   num_devices=NCORES)

    # A1P/A2P: [97, HT*256] — per h-tile block of (hi 128 | lo 128) bf16 cols
    dVR = nc.dram_tensor("VR", (1, 2 * NLOC), bf16, kind="ExternalInput")
    dA1 = nc.dram_tensor("A1P", (P, 2 * H), bf16, kind="ExternalInput")
    dA2 = nc.dram_tensor("A2P", (P, 2 * H), bf16, kind="ExternalInput")
    dW2 = nc.dram_tensor("W2TP", (128, HT * P), f16, kind="ExternalInput")
    dQB = nc.dram_tensor("QBB", (128, 2 * HT), f32, kind="ExternalInput")
    dB2E = nc.dram_tensor("B2E", (P, 1), f32, kind="ExternalInput")
    dOUT = nc.dram_tensor("out", (P, NLOC), f16, kind="ExternalOutput")

    b = float(beta)
    qb_scale = float(np.float32(-b / ((1.0 - b) * thr)))   # beta*kq
    c_open = float(np.float32(-b * b / (1.0 - b)))
    b2c = float(np.float32(b * b))
    uq_scale = float(np.float32(-(b ** 15) * qb_scale))

    with TileContext(nc) as tc:
        with tc.tile_pool(name="const", bufs=1) as cpool, \
             tc.tile_pool(name="work", bufs=2) as wpool, \
             tc.tile_pool(name="psA", bufs=3, space="PSUM") as psa_pool, \
             tc.tile_pool(name="psO", bufs=1, space="PSUM") as pso_pool:

            # ---- constants into SBUF (spread across DMA rings) ----
            tVR = cpool.tile([1, 2 * NLOC], bf16, tag="tVR")
            tA1 = cpool.tile([P, 2 * H], bf16, tag="tA1")
            tA2 = cpool.tile([P, 2 * H], bf16, tag="tA2")
            tW2 = cpool.tile([128, HT * P], f16, tag="tW2")
            tQB = cpool.tile([128, 2 * HT], f32, tag="tQB")
            tQBn = tQB[:, HT:2 * HT]
            tB2E = cpool.tile([P, 1], f32, tag="tB2E")
            # per-tile A slices alternate the two HWDGE rings, tile 0 first
            # (DMA completion semaphores lag data by ~6-10us; order matters)
            nc.sync.dma_start(out=tVR, in_=dVR.ap())
            nc.scalar.dma_start(out=tA2[:, 0:256], in_=dA2.ap()[:, 0:256])
            nc.sync.dma_start(out=tA1[:, 0:256], in_=dA1.ap()[:, 0:256])
            nc.sync.dma_start(out=tQB, in_=dQB.ap())
            for i in range(1, HT):
                sl = slice(i * 256, (i + 1) * 256)
                nc.sync.dma_start(out=tA1[:, sl], in_=dA1.ap()[:, sl])
                nc.scalar.dma_start(out=tA2[:, sl], in_=dA2.ap()[:, sl])
            nc.scalar.dma_start(out=tW2, in_=dW2.ap())
            nc.scalar.dma_start(out=tB2E, in_=dB2E.ap())

            tONE = cpool.tile([1, P], bf16, tag="tONE")
            tONL = cpool.tile([1, NLOC], bf16, tag="tONL")
            tNIO = cpool.tile([1, P], bf16, tag="tNIO")
            nc.vector.memset(tONE, 1.0)
            nc.vector.memset(tONL, 1.0)
            # negated partition-index row on device (no DMA, no receipt wait)
            nc.vector._custom_dve(ops["SNN_NIOTA"], out=tNIO, in0=tONE, s0=0.0)

            # ---- one-hot O1/O2 generated on device (bf16) ----
            # ps[p, k] = v0[k] - p  via two rank-1 matmuls, then is_eq 0
            tO1 = cpool.tile([P, NLOC], bf16, tag="tO1")
            tO2 = cpool.tile([P, NLOC], bf16, tag="tO2")
            for (c0, cw) in CHUNKS:
                for r, tO in ((0, tO1), (1, tO2)):
                    ps = psa_pool.tile([128, 512], f32, tag="psA")
                    nc.tensor.matmul(ps[0:P, 0:cw], tONE,
                                     tVR[0:1, r * NLOC + c0:r * NLOC + c0 + cw],
                                     start=True, stop=False)
                    nc.tensor.matmul(ps[0:P, 0:cw], tNIO,
                                     tONL[0:1, c0:c0 + cw],
                                     start=False, stop=True)
                    nc.vector.tensor_scalar(tO[:, c0:c0 + cw], ps[0:P, 0:cw],
                                            0.0, None,
                                            Alu.is_equal, Alu.bypass)

            # persistent output psum banks, accumulated across hidden tiles
            pso = [pso_pool.tile([P, cw], f32, tag=f"pso{n}", name=f"pso{n}")
                   for n, (c0, cw) in enumerate(CHUNKS)]

            for i in range(HT):
                hi = slice(i * 256, i * 256 + 128)
                lo = slice(i * 256 + 128, (i + 1) * 256)

                # ---- stage A: cur1 psum (bf16 hi/lo one-hot gather) ----
                qb = wpool.tile([128, NLOC], f32, tag="qb")
                uq = wpool.tile([128, NLOC], f16, tag="uq")
                for (c0, cw) in CHUNKS:
                    ps = psa_pool.tile([128, 512], f32, tag="psA")
                    nc.tensor.matmul(ps[:, 0:cw], tA1[:, hi],
                                     tO1[:, c0:c0 + cw], start=True, stop=False)
                    nc.tensor.matmul(ps[:, 0:cw], tA1[:, lo],
                                     tO1[:, c0:c0 + cw], start=False, stop=False)
                    nc.tensor.matmul(ps[:, 0:cw], tA2[:, hi],
                                     tO2[:, c0:c0 + cw], start=False, stop=False)
                    nc.tensor.matmul(ps[:, 0:cw], tA2[:, lo],
                                     tO2[:, c0:c0 + cw], start=False, stop=True)
                    # qb = beta*kq*cur1_nob1 + beta*(1 + kq*b1)  on ScalarE
                    nc.scalar.activation(qb[:, c0:c0 + cw], ps[:, 0:cw],
                                         Act.Identity, bias=tQB[:, i:i + 1],
                                         scale=qb_scale)
                    # uq = -beta^15*qb (fp16), second rhs for stage D
                    nc.scalar.activation(uq[:, c0:c0 + cw], ps[:, 0:cw],
                                         Act.Identity, bias=tQBn[:, i:i + 1],
                                         scale=uq_scale)

                # ---- stage B: 15-step spike recurrence, 6 fused DVE ops ----
                st = wpool.tile([128, NLOC], f32, tag="st")
                z16 = wpool.tile([128, NLOC], f16, tag="z16")
                nc.vector._custom_dve(ops["SNN_OPEN"], out=st, in0=qb,
                                      s0=c_open, s1=b, imm2=b2c)
                nc.vector._custom_dve(ops["SNN_WGZ"], out=st, in0=st, in1=qb, s0=b)
                nc.vector._custom_dve(ops["SNN_ZGW"], out=st, in0=st, in1=qb, s0=b)
                nc.vector._custom_dve(ops["SNN_WGZ"], out=st, in0=st, in1=qb, s0=b)
                nc.vector._custom_dve(ops["SNN_ZGW"], out=st, in0=st, in1=qb, s0=b)
                nc.vector._custom_dve(ops["SNN_WGZ"], out=z16, in0=st, in1=qb, s0=b)

                # ---- stage D: out psum += (-W2_i).T @ (z16 + uq)  ----
                # out = W2(beta^15*qb - z16) = (-W2)@z16 + (-W2)@(-beta^15*qb)
                for n, (c0, cw) in enumerate(CHUNKS):
                    nc.tensor.matmul(pso[n], tW2[:, i * P:(i + 1) * P],
                                     z16[:, c0:c0 + cw],
                                     start=(i == 0), stop=False)
                    nc.tensor.matmul(pso[n], tW2[:, i * P:(i + 1) * P],
                                     uq[:, c0:c0 + cw],
                                     start=False, stop=(i == HT - 1))

            # ---- epilogue: add b2_eff on ScalarE, store (fp16, both rings) ----
            for n, (c0, cw) in enumerate(CHUNKS):
                ob = wpool.tile([P, cw], f16, tag=f"ob{n}")
                nc.scalar.activation(ob, pso[n], Act.Identity,
                                     bias=tB2E[:, 0:1], scale=1.0)
                eng = nc.sync if n % 2 == 0 else nc.scalar
                eng.dma_start(out=dOUT.ap()[:, c0:c0 + cw], in_=ob)

    if not nc.is_finalized():
        nc.finalize()
    return nc


def _to_bf16(a):
    import ml_dtypes
    return np.asarray(a, np.float32).astype(ml_dtypes.bfloat16)


def _prep_inputs(x, embed_w, W1, b1, W2, b2, beta, thr):
    E = embed_w.astype(np.float64)
    W1d = W1.astype(np.float64)
    A1T = np.ascontiguousarray(E @ W1d[:, :H].T).astype(np.float32)   # [97, 512]
    A2T = np.ascontiguousarray(E @ W1d[:, H:].T).astype(np.float32)

    def hi_lo_pack(A):
        # [97, HT*256]: per h-tile block of (hi 128 | lo 128) columns
        hi = _to_bf16(A).reshape(P, HT, 128)
        lo = _to_bf16(A - hi.astype(np.float32).reshape(P, H)).reshape(P, HT, 128)
        return np.ascontiguousarray(
            np.concatenate([hi, lo], axis=2).reshape(P, 2 * H))

    A1P = hi_lo_pack(A1T)   # [97, 1024] bf16
    A2P = hi_lo_pack(A2T)

    pid = np.arange(NPAD)
    v0 = (pid // P).astype(np.float32)
    v1 = (pid % P).astype(np.float32)

    kq = -1.0 / ((1.0 - beta) * thr)
    qbias = (beta * (1.0 + kq * b1.astype(np.float64)))
    QBB = np.ascontiguousarray(np.concatenate([
        qbias.astype(np.float32).reshape(HT, 128).T,
        (-(beta ** 15) * qbias).astype(np.float32).reshape(HT, 128).T,
    ], axis=1))

    G = (1.0 - beta ** NSTEPS) / (1.0 - beta)
    b2e = (G * b2.astype(np.float64)
           - (beta ** 16) * W2.astype(np.float64).sum(axis=1))
    B2E = np.ascontiguousarray(b2e.astype(np.float32).reshape(P, 1))

    # [128, 4*97] fp16: h-tile i's NEGATED W2 block in cols [i*97, (i+1)*97)
    W2TP = np.ascontiguousarray(
        (-W2.T).astype(np.float16).reshape(HT, 128, P).transpose(1, 0, 2)
        .reshape(128, HT * P))

    in_maps = []
    for k in range(NCORES):
        sl = slice(k * NLOC, (k + 1) * NLOC)
        VR = _to_bf16(np.concatenate([v0[sl], v1[sl]]).reshape(1, 2 * NLOC))
        in_maps.append({
            "VR": np.ascontiguousarray(VR),
            "A1P": A1P, "A2P": A2P, "W2TP": W2TP, "QBB": QBB, "B2E": B2E,
        })
    return in_maps


def kernel(x, embed_w, W1, b1, W2, b2, beta1, beta2, thr1, thr2, **_):
    from concourse.bass_utils import run_bass_kernel_spmd

    beta = float(np.clip(np.float32(beta1), 0.1, 0.9))
    beta2c = float(np.clip(np.float32(beta2), 0.1, 0.9))
    thr = float(max(np.float32(thr1), 0.1))
    assert abs(beta - beta2c) < 1e-12, "kernel assumes beta1 == beta2"

    key = (round(beta, 9), round(thr, 9))
    if key not in _CACHE:
        _CACHE[key] = _build_bass(beta, thr)
    nc = _CACHE[key]

    in_maps = _prep_inputs(x, embed_w, W1, b1, W2, b2, beta, thr)
    res = run_bass_kernel_spmd(nc, in_maps, core_ids=list(range(NCORES)))
    T = np.concatenate([r["out"].astype(np.float32) for r in res.results],
                       axis=1)[:, :NGRID]

    pid = x[:, 0].astype(np.int64) * P + x[:, 1].astype(np.int64)
    return np.ascontiguousarray(T.T[pid]).astype(np.float32)


# revision 38
# speedup vs baseline: 1.1346x; 1.0042x over previous
"""GrokkingSNN forward on 8 TRN2 NeuronCores.

Math notes
----------
Reference loop (thr=1 after clamps, beta1=beta2=beta):
    m_t = beta*m_{t-1} + c - s_{t-1},  s_t = H(m_t - 1),  m_0 = 0, s_0 = 0
    out = W2 @ S + G*b2,  S = sum_t beta^(15-t) s_t,  G = (1-beta^15)/(1-beta)

With z = m - m*, m* = c/(1-beta), q = 1 - m* (so s_t = [z_t > q]) the
spike recurrence telescopes against the output weights:
    S = beta^16*q - z16 - beta^16,   z16 = beta*z15 - s15
so the device only needs z16.  The -beta^16 constant folds into b2.

x has only 97*97 = 9409 distinct rows, so the network is evaluated once
per distinct (x0, x1) pair on a fixed grid; the final answer is a row
gather by pair id.  cur1 for the grid is built on-device from one-hot
matmuls (the one-hots are generated on device from the v0/v1 index rows)
against the embedding-folded weights A1/A2, split hi/lo in bf16 so the
PE gather stays exact to ~2^-17 while running at bf16 rate.

The 15-step spike recurrence runs as SIX fused custom-DVE ops per
hidden tile (46 ALU stages total) on the premultiplied state w = beta*z
and threshold qb = beta*q:
    OPEN(qb)   -> w3    (s1 via a constant compare, then 2 full steps)
    WGZ(w,qb)  -> z+3   (3 spike decisions, 8 ALUs)
    ZGW(z,qb)  -> w+2   (2 spike decisions, 7 ALUs)
chained OPEN,WGZ,ZGW,WGZ,ZGW,WGZ = z16.  S~ = beta^15*qb - z16 via one
scalar_tensor_tensor.  qb itself is produced on the Scalar engine
(activation Identity with per-partition bias) straight from PSUM.
"""

import os
import sys

import numpy as np

for _p in ("/opt/trn_rl_repo",):
    if _p not in sys.path and os.path.isdir(_p):
        sys.path.insert(0, _p)

P = 97          # vocab / output dim
H = 512         # hidden
NSTEPS = 15
NCORES = 8
NGRID = P * P   # 9409 distinct input pairs
NPAD = 9472     # 8 * 1184
NLOC = NPAD // NCORES   # 1184 grid columns per core
HT = H // 128   # 4 hidden tiles
CHUNKS = [(0, 512), (512, 512), (1024, NLOC - 1024)]  # psum-sized col chunks

_CACHE = {}
_DVE_OPS = {}


def _register_dve_ops():
    """Register the three fused recurrence ops (documented extension
    path: define a DveOp and append to dve_ops.OPS)."""
    if _DVE_OPS:
        return _DVE_OPS
    import concourse.dve_ops as dvo
    from concourse.dve_spec import (Spec, Src0, Src1, C0, C1, C2, Idx, Zero,
                                    lower, _has_src1)
    from concourse.dve_uop import DveOpSpec

    def f32(a):
        return np.asarray(a, np.float32)

    # NIOTA: out[p, k] = -k  (negated element index; Src0 ignored)
    NIOTA_spec = Spec(
        body=(Zero - Idx) + Src0 * C0,
        reference=lambda in0, in1, s0, s1, imm2: (
            -np.broadcast_to(np.arange(in0.shape[-1], dtype=np.float32),
                             in0.shape) + in0 * s0).astype(np.float32),
    )

    # OPEN: in0 = qb (= beta*q).  s1 = [qb < c0]; z2 = qb*c1 - c2 - s1;
    # w2 = z2*c1; s2 = [w2 > qb]; z3 = w2 - s2; out = z3*c1   (c1 = beta)
    _s1 = Src0 < C0
    _z2 = (Src0 * C1 - C2) - _s1
    _w2 = _z2 * C1
    _z3 = _w2 - (_w2 > Src0)
    OPEN_spec = Spec(
        body=_z3 * C1,
        reference=lambda in0, in1, s0, s1, imm2: (lambda qb, s1v: (
            lambda z2: (lambda w2: (lambda z3: f32(z3 * s1))(
                f32(w2 - (w2 > qb))))(f32(z2 * s1)))(
            f32(f32(f32(qb * s1) - imm2) - s1v)))(
            f32(in0), f32(in0 < s0)),
    )

    # WGZ: in0 = w_t, in1 = qb: three spike decisions -> z_{t+3}
    _a2 = Src0 - (Src0 > Src1)
    _a3 = _a2 * C0
    _a5 = _a3 - (_a3 > Src1)
    _a6 = _a5 * C0
    WGZ_spec = Spec(
        body=_a6 - (_a6 > Src1),
        reference=lambda in0, in1, s0, s1, imm2: (lambda w, qb, b: (
            lambda z1: (lambda w1: (lambda z2: (lambda w2: f32(w2 - (w2 > qb)))(
                f32(z2 * b)))(f32(w1 - (w1 > qb))))(f32(z1 * b)))(
            f32(w - (w > qb))))(f32(in0), f32(in1), s0),
    )

    # ZGW: in0 = z_t, in1 = qb: two spike decisions -> w_{t+2}
    _b1 = Src0 * C0
    _b3 = _b1 - (_b1 > Src1)
    _b4 = _b3 * C0
    _b6 = _b4 - (_b4 > Src1)
    ZGW_spec = Spec(
        body=_b6 * C0,
        reference=lambda in0, in1, s0, s1, imm2: (lambda z, qb, b: (
            lambda w: (lambda z1: (lambda w1: (lambda z2: f32(z2 * b))(
                f32(w1 - (w1 > qb))))(f32(z1 * b)))(f32(w - (w > qb))))(
            f32(z * b)))(f32(in0), f32(in1), s0),
    )

    for name, spec in (("SNN_OPEN", OPEN_spec), ("SNN_WGZ", WGZ_spec),
                       ("SNN_ZGW", ZGW_spec), ("SNN_NIOTA", NIOTA_spec)):
        if name not in dvo._SUB_OPCODE_FOR_NAME:
            row = dvo._CUSTOM_DVE_ROW_BASE + len(dvo.OPS)
            assert row < 0x20
            dvo._SUB_OPCODE_FOR_NAME[name] = row
            shas = {}
            for ver in ("v3", "v4"):
                uops = lower(spec, ver=ver)
                shas[ver] = DveOpSpec(name=name, opcode=row, uops=uops,
                                      rd1_en=_has_src1(spec)).sha(ver)
            # OPEN is single-stream -> eligible for the 2x_2P fp32 perf mode
            perf = {"v3": True, "v4": True} if name == "SNN_OPEN" else {}
            op = dvo.DveOp(name, spec, subdim=False, uops_sha=shas,
                           perf_en=perf)
            dvo.OPS.append(op)
            dvo.CUSTOM_DVE_SPECS[name] = spec
        _DVE_OPS[name] = next(o for o in dvo.OPS if o.name == name)
    return _DVE_OPS


def _build_bass(beta: float, thr: float):
    from concourse import bacc, mybir
    from concourse.tile import TileContext

    ops = _register_dve_ops()
    f32 = mybir.dt.float32
    bf16 = mybir.dt.bfloat16
    f16 = mybir.dt.float16
    Alu = mybir.AluOpType
    Act = mybir.ActivationFunctionType

    nc = bacc.Bacc("TRN2", target_bir_lowering=False, debug=False,
                   num_devices=NCORES)

    # A1P/A2P: [97, HT*256] — per h-tile block of (hi 128 | lo 128) bf16 cols
    dVR = nc.dram_tensor("VR", (1, 2 * NLOC), bf16, kind="ExternalInput")
    dA1 = nc.dram_tensor("A1P", (P, 2 * H), bf16, kind="ExternalInput")
    dA2 = nc.dram_tensor("A2P", (P, 2 * H), bf16, kind="ExternalInput")
    dW2 = nc.dram_tensor("W2TP", (128, HT * P), f16, kind="ExternalInput")
    # QBB: [128, 2*HT+1] — qb bias | uq bias | b2_eff (b2e in rows 0..96)
    dQB = nc.dram_tensor("QBB", (128, 2 * HT + 1), f32, kind="ExternalInput")
    dOUT = nc.dram_tensor("out", (P, NLOC), f16, kind="ExternalOutput")

    b = float(beta)
    qb_scale = float(np.float32(-b / ((1.0 - b) * thr)))   # beta*kq
    c_open = float(np.float32(-b * b / (1.0 - b)))
    b2c = float(np.float32(b * b))
    uq_scale = float(np.float32(-(b ** 15) * qb_scale))

    with TileContext(nc) as tc:
        with tc.tile_pool(name="const", bufs=1) as cpool, \
             tc.tile_pool(name="work", bufs=2) as wpool, \
             tc.tile_pool(name="psA", bufs=3, space="PSUM") as psa_pool, \
             tc.tile_pool(name="psO", bufs=1, space="PSUM") as pso_pool:

            # ---- constants into SBUF (spread across DMA rings) ----
            tVR = cpool.tile([1, 2 * NLOC], bf16, tag="tVR")
            tA1 = cpool.tile([P, 2 * H], bf16, tag="tA1")
            tA2 = cpool.tile([P, 2 * H], bf16, tag="tA2")
            tW2 = cpool.tile([128, HT * P], f16, tag="tW2")
            tQB = cpool.tile([128, 2 * HT + 1], f32, tag="tQB")
            tQBn = tQB[:, HT:2 * HT]
            tB2E = tQB[0:P, 2 * HT:2 * HT + 1]
            # per-tile A slices alternate the two HWDGE rings, tile 0 first
            # (DMA completion semaphores lag data by ~6-10us; order matters)
            nc.scalar.dma_start(out=tA2[:, 0:256], in_=dA2.ap()[:, 0:256])
            nc.sync.dma_start(out=tA1[:, 0:256], in_=dA1.ap()[:, 0:256])
            nc.sync.dma_start(out=tVR, in_=dVR.ap())
            nc.sync.dma_start(out=tQB, in_=dQB.ap())
            for i in range(1, HT):
                sl = slice(i * 256, (i + 1) * 256)
                nc.sync.dma_start(out=tA1[:, sl], in_=dA1.ap()[:, sl])
                nc.scalar.dma_start(out=tA2[:, sl], in_=dA2.ap()[:, sl])
            nc.scalar.dma_start(out=tW2, in_=dW2.ap())

            tONE = cpool.tile([1, P], bf16, tag="tONE")
            tNIO = cpool.tile([1, P], bf16, tag="tNIO")
            tONE1 = cpool.tile([1, 1], bf16, tag="tONE1")
            tPX = cpool.tile([P, 1], f32, tag="tPX")
            nc.vector.memset(tONE, 1.0)
            nc.vector.memset(tONE1, 1.0)
            # negated partition-index row on device (no DMA, no receipt wait)
            nc.vector._custom_dve(ops["SNN_NIOTA"], out=tNIO, in0=tONE, s0=0.0)
            # transpose it into a per-partition column via a rank-1 matmul
            psx = psa_pool.tile([128, 512], f32, tag="psA")
            nc.tensor.matmul(psx[0:P, 0:1], tNIO, tONE1, start=True, stop=True)
            nc.scalar.activation(tPX, psx[0:P, 0:1], Act.Identity,
                                 bias=0.0, scale=-1.0)

            # ---- one-hot O1/O2 generated on device (bf16) ----
            tO1 = cpool.tile([P, NLOC], bf16, tag="tO1")
            tO2 = cpool.tile([P, NLOC], bf16, tag="tO2")
            for (c0, cw) in CHUNKS:
                for r, tO in ((0, tO1), (1, tO2)):
                    ps = psa_pool.tile([128, 512], f32, tag="psA")
                    nc.tensor.matmul(ps[0:P, 0:cw], tONE,
                                     tVR[0:1, r * NLOC + c0:r * NLOC + c0 + cw],
                                     start=True, stop=True)
                    nc.vector.tensor_scalar(tO[:, c0:c0 + cw], ps[0:P, 0:cw],
                                            tPX[:, 0:1], None,
                                            Alu.is_equal, Alu.bypass)

            # persistent output psum banks, accumulated across hidden tiles
            pso = [pso_pool.tile([P, cw], f32, tag=f"pso{n}", name=f"pso{n}")
                   for n, (c0, cw) in enumerate(CHUNKS)]

            for i in range(HT):
                hi = slice(i * 256, i * 256 + 128)
                lo = slice(i * 256 + 128, (i + 1) * 256)

                # ---- stage A: cur1 psum (bf16 hi/lo one-hot gather) ----
                qb = wpool.tile([128, NLOC], f32, tag="qb")
                uq = wpool.tile([128, NLOC], f16, tag="uq")
                pss = []
                for (c0, cw) in CHUNKS:
                    ps = psa_pool.tile([128, 512], f32, tag="psA")
                    pss.append(ps)
                    nc.tensor.matmul(ps[:, 0:cw], tA1[:, hi],
                                     tO1[:, c0:c0 + cw], start=True, stop=False)
                    nc.tensor.matmul(ps[:, 0:cw], tA1[:, lo],
                                     tO1[:, c0:c0 + cw], start=False, stop=False)
                    nc.tensor.matmul(ps[:, 0:cw], tA2[:, hi],
                                     tO2[:, c0:c0 + cw], start=False, stop=False)
                    nc.tensor.matmul(ps[:, 0:cw], tA2[:, lo],
                                     tO2[:, c0:c0 + cw], start=False, stop=True)
                    # qb = beta*kq*cur1_nob1 + beta*(1 + kq*b1)  on ScalarE
                    nc.scalar.activation(qb[:, c0:c0 + cw], ps[:, 0:cw],
                                         Act.Identity, bias=tQB[:, i:i + 1],
                                         scale=qb_scale)
                # uq = -beta^15*qb (fp16), second rhs for stage D — emitted
                # after all qb chunks so the DVE chain starts sooner
                for (c0, cw), ps in zip(CHUNKS, pss):
                    nc.scalar.activation(uq[:, c0:c0 + cw], ps[:, 0:cw],
                                         Act.Identity, bias=tQBn[:, i:i + 1],
                                         scale=uq_scale)

                # ---- stage B: 15-step spike recurrence, 6 fused DVE ops ----
                st = wpool.tile([128, NLOC], f32, tag="st")
                z16 = wpool.tile([128, NLOC], f16, tag="z16")
                nc.vector._custom_dve(ops["SNN_OPEN"], out=st, in0=qb,
                                      s0=c_open, s1=b, imm2=b2c)
                nc.vector._custom_dve(ops["SNN_WGZ"], out=st, in0=st, in1=qb, s0=b)
                nc.vector._custom_dve(ops["SNN_ZGW"], out=st, in0=st, in1=qb, s0=b)
                nc.vector._custom_dve(ops["SNN_WGZ"], out=st, in0=st, in1=qb, s0=b)
                nc.vector._custom_dve(ops["SNN_ZGW"], out=st, in0=st, in1=qb, s0=b)
                nc.vector._custom_dve(ops["SNN_WGZ"], out=z16, in0=st, in1=qb, s0=b)

                # ---- stage D: out psum += (-W2_i).T @ (z16 + uq)  ----
                # out = W2(beta^15*qb - z16) = (-W2)@z16 + (-W2)@(-beta^15*qb)
                for n, (c0, cw) in enumerate(CHUNKS):
                    nc.tensor.matmul(pso[n], tW2[:, i * P:(i + 1) * P],
                                     z16[:, c0:c0 + cw],
                                     start=(i == 0), stop=False)
                    nc.tensor.matmul(pso[n], tW2[:, i * P:(i + 1) * P],
                                     uq[:, c0:c0 + cw],
                                     start=False, stop=(i == HT - 1))

            # ---- epilogue: add b2_eff on ScalarE, store (fp16, both rings) ----
            for n, (c0, cw) in enumerate(CHUNKS):
                ob = wpool.tile([P, cw], f16, tag=f"ob{n}")
                nc.scalar.activation(ob, pso[n], Act.Identity,
                                     bias=tB2E[:, 0:1], scale=1.0)
                eng = nc.sync if n % 2 == 0 else nc.scalar
                eng.dma_start(out=dOUT.ap()[:, c0:c0 + cw], in_=ob)

    if not nc.is_finalized():
        nc.finalize()
    return nc


def _to_bf16(a):
    import ml_dtypes
    return np.asarray(a, np.float32).astype(ml_dtypes.bfloat16)


def _prep_inputs(x, embed_w, W1, b1, W2, b2, beta, thr):
    E = embed_w.astype(np.float64)
    W1d = W1.astype(np.float64)
    A1T = np.ascontiguousarray(E @ W1d[:, :H].T).astype(np.float32)   # [97, 512]
    A2T = np.ascontiguousarray(E @ W1d[:, H:].T).astype(np.float32)

    def hi_lo_pack(A):
        # [97, HT*256]: per h-tile block of (hi 128 | lo 128) columns
        hi = _to_bf16(A).reshape(P, HT, 128)
        lo = _to_bf16(A - hi.astype(np.float32).reshape(P, H)).reshape(P, HT, 128)
        return np.ascontiguousarray(
            np.concatenate([hi, lo], axis=2).reshape(P, 2 * H))

    A1P = hi_lo_pack(A1T)   # [97, 1024] bf16
    A2P = hi_lo_pack(A2T)

    pid = np.arange(NPAD)
    v0 = (pid // P).astype(np.float32)
    v1 = (pid % P).astype(np.float32)

    kq = -1.0 / ((1.0 - beta) * thr)
    qbias = (beta * (1.0 + kq * b1.astype(np.float64)))

    G = (1.0 - beta ** NSTEPS) / (1.0 - beta)
    b2e = (G * b2.astype(np.float64)
           - (beta ** 16) * W2.astype(np.float64).sum(axis=1))
    b2e_col = np.zeros((128, 1), np.float32)
    b2e_col[:P, 0] = b2e.astype(np.float32)

    # [128, 4*97] fp16: h-tile i's NEGATED W2 block in cols [i*97, (i+1)*97)
    W2TP = np.ascontiguousarray(
        (-W2.T).astype(np.float16).reshape(HT, 128, P).transpose(1, 0, 2)
        .reshape(128, HT * P))

    QBB = np.ascontiguousarray(np.concatenate([
        qbias.astype(np.float32).reshape(HT, 128).T,
        (-(beta ** 15) * qbias).astype(np.float32).reshape(HT, 128).T,
        b2e_col,
    ], axis=1))

    in_maps = []
    for k in range(NCORES):
        sl = slice(k * NLOC, (k + 1) * NLOC)
        VR = _to_bf16(np.concatenate([v0[sl], v1[sl]]).reshape(1, 2 * NLOC))
        in_maps.append({
            "VR": np.ascontiguousarray(VR),
            "A1P": A1P, "A2P": A2P, "W2TP": W2TP, "QBB": QBB,
        })
    return in_maps


def kernel(x, embed_w, W1, b1, W2, b2, beta1, beta2, thr1, thr2, **_):
    from concourse.bass_utils import run_bass_kernel_spmd

    beta = float(np.clip(np.float32(beta1), 0.1, 0.9))
    beta2c = float(np.clip(np.float32(beta2), 0.1, 0.9))
    thr = float(max(np.float32(thr1), 0.1))
    assert abs(beta - beta2c) < 1e-12, "kernel assumes beta1 == beta2"

    key = (round(beta, 9), round(thr, 9))
    if key not in _CACHE:
        _CACHE[key] = _build_bass(beta, thr)
    nc = _CACHE[key]

    in_maps = _prep_inputs(x, embed_w, W1, b1, W2, b2, beta, thr)
    res = run_bass_kernel_spmd(nc, in_maps, core_ids=list(range(NCORES)))
    T = np.concatenate([r["out"].astype(np.float32) for r in res.results],
                       axis=1)[:, :NGRID]

    pid = x[:, 0].astype(np.int64) * P + x[:, 1].astype(np.int64)
    return np.ascontiguousarray(T.T[pid]).astype(np.float32)


# revision 41
# speedup vs baseline: 1.1574x; 1.0201x over previous
"""GrokkingSNN forward on 8 TRN2 NeuronCores.

Math notes
----------
Reference loop (thr=1 after clamps, beta1=beta2=beta):
    m_t = beta*m_{t-1} + c - s_{t-1},  s_t = H(m_t - 1),  m_0 = 0, s_0 = 0
    out = W2 @ S + G*b2,  S = sum_t beta^(15-t) s_t,  G = (1-beta^15)/(1-beta)

With z = m - m*, m* = c/(1-beta), q = 1 - m* (so s_t = [z_t > q]) the
spike recurrence telescopes against the output weights:
    S = beta^16*q - z16 - beta^16,   z16 = beta*z15 - s15
so the device only needs z16.  The -beta^16 constant folds into b2.

x has only 97*97 = 9409 distinct rows, so the network is evaluated once
per distinct (x0, x1) pair on a fixed grid; the final answer is a row
gather by pair id.  cur1 for the grid is built on-device from one-hot
matmuls (the one-hots are generated on device from the v0/v1 index rows)
against the embedding-folded weights A1/A2, split hi/lo in bf16 so the
PE gather stays exact to ~2^-17 while running at bf16 rate.

The 15-step spike recurrence runs as SIX fused custom-DVE ops per
hidden tile (46 ALU stages total) on the premultiplied state w = beta*z
and threshold qb = beta*q:
    OPEN(qb)   -> w3    (s1 via a constant compare, then 2 full steps)
    WGZ(w,qb)  -> z+3   (3 spike decisions, 8 ALUs)
    ZGW(z,qb)  -> w+2   (2 spike decisions, 7 ALUs)
chained OPEN,WGZ,ZGW,WGZ,ZGW,WGZ = z16.  S~ = beta^15*qb - z16 via one
scalar_tensor_tensor.  qb itself is produced on the Scalar engine
(activation Identity with per-partition bias) straight from PSUM.
"""

import os
import sys

import numpy as np

for _p in ("/opt/trn_rl_repo",):
    if _p not in sys.path and os.path.isdir(_p):
        sys.path.insert(0, _p)

P = 97          # vocab / output dim
H = 512         # hidden
NSTEPS = 15
NCORES = 8
NGRID = P * P   # 9409 distinct input pairs
NPAD = 9472     # 8 * 1184
NLOC = NPAD // NCORES   # 1184 grid columns per core
HT = H // 128   # 4 hidden tiles
CHUNKS = [(0, 512), (512, 512), (1024, NLOC - 1024)]  # psum-sized col chunks

_CACHE = {}
_DVE_OPS = {}


def _register_dve_ops():
    """Register the three fused recurrence ops (documented extension
    path: define a DveOp and append to dve_ops.OPS)."""
    if _DVE_OPS:
        return _DVE_OPS
    import concourse.dve_ops as dvo
    from concourse.dve_spec import (Spec, Src0, Src1, C0, C1, C2, Idx, Zero,
                                    lower, _has_src1)
    from concourse.dve_uop import DveOpSpec

    def f32(a):
        return np.asarray(a, np.float32)

    # NIOTA: out[p, k] = -k  (negated element index; Src0 ignored)
    NIOTA_spec = Spec(
        body=(Zero - Idx) + Src0 * C0,
        reference=lambda in0, in1, s0, s1, imm2: (
            -np.broadcast_to(np.arange(in0.shape[-1], dtype=np.float32),
                             in0.shape) + in0 * s0).astype(np.float32),
    )

    # OPEN: in0 = qb (= beta*q).  s1 = [qb < c0]; z2 = qb*c1 - c2 - s1;
    # w2 = z2*c1; s2 = [w2 > qb]; z3 = w2 - s2; out = z3*c1   (c1 = beta)
    _s1 = Src0 < C0
    _z2 = (Src0 * C1 - C2) - _s1
    _w2 = _z2 * C1
    _z3 = _w2 - (_w2 > Src0)
    OPEN_spec = Spec(
        body=_z3 * C1,
        reference=lambda in0, in1, s0, s1, imm2: (lambda qb, s1v: (
            lambda z2: (lambda w2: (lambda z3: f32(z3 * s1))(
                f32(w2 - (w2 > qb))))(f32(z2 * s1)))(
            f32(f32(f32(qb * s1) - imm2) - s1v)))(
            f32(in0), f32(in0 < s0)),
    )

    # WGZ: in0 = w_t, in1 = qb: three spike decisions -> z_{t+3}
    _a2 = Src0 - (Src0 > Src1)
    _a3 = _a2 * C0
    _a5 = _a3 - (_a3 > Src1)
    _a6 = _a5 * C0
    WGZ_spec = Spec(
        body=_a6 - (_a6 > Src1),
        reference=lambda in0, in1, s0, s1, imm2: (lambda w, qb, b: (
            lambda z1: (lambda w1: (lambda z2: (lambda w2: f32(w2 - (w2 > qb)))(
                f32(z2 * b)))(f32(w1 - (w1 > qb))))(f32(z1 * b)))(
            f32(w - (w > qb))))(f32(in0), f32(in1), s0),
    )

    # ZGW: in0 = z_t, in1 = qb: two spike decisions -> w_{t+2}
    _b1 = Src0 * C0
    _b3 = _b1 - (_b1 > Src1)
    _b4 = _b3 * C0
    _b6 = _b4 - (_b4 > Src1)
    ZGW_spec = Spec(
        body=_b6 * C0,
        reference=lambda in0, in1, s0, s1, imm2: (lambda z, qb, b: (
            lambda w: (lambda z1: (lambda w1: (lambda z2: f32(z2 * b))(
                f32(w1 - (w1 > qb))))(f32(z1 * b)))(f32(w - (w > qb))))(
            f32(z * b)))(f32(in0), f32(in1), s0),
    )

    for name, spec in (("SNN_OPEN", OPEN_spec), ("SNN_WGZ", WGZ_spec),
                       ("SNN_ZGW", ZGW_spec), ("SNN_NIOTA", NIOTA_spec)):
        if name not in dvo._SUB_OPCODE_FOR_NAME:
            row = dvo._CUSTOM_DVE_ROW_BASE + len(dvo.OPS)
            assert row < 0x20
            dvo._SUB_OPCODE_FOR_NAME[name] = row
            shas = {}
            for ver in ("v3", "v4"):
                uops = lower(spec, ver=ver)
                shas[ver] = DveOpSpec(name=name, opcode=row, uops=uops,
                                      rd1_en=_has_src1(spec)).sha(ver)
            # OPEN is single-stream -> eligible for the 2x_2P fp32 perf mode
            perf = {"v3": True, "v4": True} if name == "SNN_OPEN" else {}
            op = dvo.DveOp(name, spec, subdim=False, uops_sha=shas,
                           perf_en=perf)
            dvo.OPS.append(op)
            dvo.CUSTOM_DVE_SPECS[name] = spec
        _DVE_OPS[name] = next(o for o in dvo.OPS if o.name == name)
    return _DVE_OPS


def _build_bass(beta: float, thr: float):
    from concourse import bacc, mybir
    from concourse.tile import TileContext

    ops = _register_dve_ops()
    f32 = mybir.dt.float32
    bf16 = mybir.dt.bfloat16
    f16 = mybir.dt.float16
    Alu = mybir.AluOpType
    Act = mybir.ActivationFunctionType

    nc = bacc.Bacc("TRN2", target_bir_lowering=False, debug=False,
                   num_devices=NCORES)

    # A1P/A2P: [97, HT*256] — per h-tile block of (hi 128 | lo 128) bf16 cols
    dVR = nc.dram_tensor("VR", (1, 2 * NLOC), bf16, kind="ExternalInput")
    dA1 = nc.dram_tensor("A1P", (P, 2 * H), bf16, kind="ExternalInput")
    dA2 = nc.dram_tensor("A2P", (P, 2 * H), bf16, kind="ExternalInput")
    dW2 = nc.dram_tensor("W2TP", (128, HT * P), f16, kind="ExternalInput")
    # QBB: [128, 2*HT+1] — qb bias | uq bias | b2_eff (b2e in rows 0..96)
    dQB = nc.dram_tensor("QBB", (128, 2 * HT + 1), f32, kind="ExternalInput")
    dOUT = nc.dram_tensor("out", (P, NLOC), f16, kind="ExternalOutput")

    b = float(beta)
    qb_scale = float(np.float32(-b / ((1.0 - b) * thr)))   # beta*kq
    c_open = float(np.float32(-b * b / (1.0 - b)))
    b2c = float(np.float32(b * b))
    uq_scale = float(np.float32(-(b ** 15) * qb_scale))

    with TileContext(nc) as tc:
        with tc.tile_pool(name="const", bufs=1) as cpool, \
             tc.tile_pool(name="work", bufs=2) as wpool, \
             tc.tile_pool(name="psA", bufs=4, space="PSUM") as psa_pool, \
             tc.tile_pool(name="psO", bufs=1, space="PSUM") as pso_pool:

            # ---- constants into SBUF (spread across DMA rings) ----
            tVR = cpool.tile([1, 2 * NLOC], bf16, tag="tVR")
            tA1 = cpool.tile([P, 2 * H], bf16, tag="tA1")
            tA2 = cpool.tile([P, 2 * H], bf16, tag="tA2")
            tW2 = cpool.tile([128, HT * P], f16, tag="tW2")
            tQB = cpool.tile([128, 2 * HT + 1], f32, tag="tQB")
            tQBn = tQB[:, HT:2 * HT]
            tB2E = tQB[0:P, 2 * HT:2 * HT + 1]
            # per-tile A slices alternate the two HWDGE rings, tile 0 first
            # (DMA completion semaphores lag data by ~6-10us; order matters)
            nc.scalar.dma_start(out=tA2[:, 0:256], in_=dA2.ap()[:, 0:256])
            nc.sync.dma_start(out=tA1[:, 0:256], in_=dA1.ap()[:, 0:256])
            nc.sync.dma_start(out=tVR, in_=dVR.ap())
            nc.sync.dma_start(out=tQB, in_=dQB.ap())
            nc.sync.dma_start(out=tA1[:, 256:512], in_=dA1.ap()[:, 256:512])
            nc.scalar.dma_start(out=tA2[:, 256:512], in_=dA2.ap()[:, 256:512])

            tONE = cpool.tile([1, P], bf16, tag="tONE")
            tNIO = cpool.tile([1, P], bf16, tag="tNIO")
            tONE1 = cpool.tile([1, 1], bf16, tag="tONE1")
            tPX = cpool.tile([P, 1], f32, tag="tPX")
            nc.vector.memset(tONE, 1.0)
            nc.vector.memset(tONE1, 1.0)
            # negated partition-index row on device (no DMA, no receipt wait)
            nc.vector._custom_dve(ops["SNN_NIOTA"], out=tNIO, in0=tONE, s0=0.0)
            # transpose it into a per-partition column via a rank-1 matmul
            psx = psa_pool.tile([128, 512], f32, tag="psA")
            nc.tensor.matmul(psx[0:P, 0:1], tNIO, tONE1, start=True, stop=True)
            nc.scalar.activation(tPX, psx[0:P, 0:1], Act.Identity,
                                 bias=0.0, scale=-1.0)

            # ---- one-hot O1/O2 generated on device (bf16) ----
            tO1 = cpool.tile([P, NLOC], bf16, tag="tO1")
            tO2 = cpool.tile([P, NLOC], bf16, tag="tO2")
            for (c0, cw) in CHUNKS:
                for r, tO in ((0, tO1), (1, tO2)):
                    ps = psa_pool.tile([128, 512], f32, tag="psA")
                    nc.tensor.matmul(ps[0:P, 0:cw], tONE,
                                     tVR[0:1, r * NLOC + c0:r * NLOC + c0 + cw],
                                     start=True, stop=True)
                    nc.vector.tensor_scalar(tO[:, c0:c0 + cw], ps[0:P, 0:cw],
                                            tPX[:, 0:1], None,
                                            Alu.is_equal, Alu.bypass)

            # persistent output psum banks, accumulated across hidden tiles
            pso = [pso_pool.tile([P, cw], f32, tag=f"pso{n}", name=f"pso{n}")
                   for n, (c0, cw) in enumerate(CHUNKS)]

            for i in range(HT):
                hi = slice(i * 256, i * 256 + 128)
                lo = slice(i * 256 + 128, (i + 1) * 256)

                if i == 0:
                    # defer far-tile loads so their SBUF writes don't
                    # contend with tile-0 stage A; receipts still arrive
                    # well before tiles 2/3 need them
                    for j in (2, 3):
                        sl = slice(j * 256, (j + 1) * 256)
                        nc.sync.dma_start(out=tA1[:, sl], in_=dA1.ap()[:, sl])
                        nc.scalar.dma_start(out=tA2[:, sl], in_=dA2.ap()[:, sl])
                    nc.scalar.dma_start(out=tW2, in_=dW2.ap())

                # ---- stage A: cur1 psum (bf16 hi/lo one-hot gather) ----
                qb = wpool.tile([128, NLOC], f32, tag="qb")
                uq = wpool.tile([128, NLOC], f16, tag="uq")
                pss = []
                for (c0, cw) in CHUNKS:
                    ps = psa_pool.tile([128, 512], f32, tag="psA")
                    pss.append(ps)
                    nc.tensor.matmul(ps[:, 0:cw], tA1[:, hi],
                                     tO1[:, c0:c0 + cw], start=True, stop=False)
                    nc.tensor.matmul(ps[:, 0:cw], tA1[:, lo],
                                     tO1[:, c0:c0 + cw], start=False, stop=False)
                    nc.tensor.matmul(ps[:, 0:cw], tA2[:, hi],
                                     tO2[:, c0:c0 + cw], start=False, stop=False)
                    nc.tensor.matmul(ps[:, 0:cw], tA2[:, lo],
                                     tO2[:, c0:c0 + cw], start=False, stop=True)
                    # qb = beta*kq*cur1_nob1 + beta*(1 + kq*b1)  on ScalarE
                    nc.scalar.activation(qb[:, c0:c0 + cw], ps[:, 0:cw],
                                         Act.Identity, bias=tQB[:, i:i + 1],
                                         scale=qb_scale)
                # uq = -beta^15*qb (fp16), second rhs for stage D — emitted
                # after all qb chunks so the DVE chain starts sooner
                for (c0, cw), ps in zip(CHUNKS, pss):
                    nc.scalar.activation(uq[:, c0:c0 + cw], ps[:, 0:cw],
                                         Act.Identity, bias=tQBn[:, i:i + 1],
                                         scale=uq_scale)

                # ---- stage B: 15-step spike recurrence, 6 fused DVE ops ----
                st = wpool.tile([128, NLOC], f32, tag="st")
                z16 = wpool.tile([128, NLOC], f16, tag="z16")
                nc.vector._custom_dve(ops["SNN_OPEN"], out=st, in0=qb,
                                      s0=c_open, s1=b, imm2=b2c)
                nc.vector._custom_dve(ops["SNN_WGZ"], out=st, in0=st, in1=qb, s0=b)
                nc.vector._custom_dve(ops["SNN_ZGW"], out=st, in0=st, in1=qb, s0=b)
                nc.vector._custom_dve(ops["SNN_WGZ"], out=st, in0=st, in1=qb, s0=b)
                nc.vector._custom_dve(ops["SNN_ZGW"], out=st, in0=st, in1=qb, s0=b)
                nc.vector._custom_dve(ops["SNN_WGZ"], out=z16, in0=st, in1=qb, s0=b)

                # ---- stage D: out psum += (-W2_i).T @ (z16 + uq)  ----
                # out = W2(beta^15*qb - z16) = (-W2)@z16 + (-W2)@(-beta^15*qb)
                for n, (c0, cw) in enumerate(CHUNKS):
                    nc.tensor.matmul(pso[n], tW2[:, i * P:(i + 1) * P],
                                     z16[:, c0:c0 + cw],
                                     start=(i == 0), stop=False)
                    nc.tensor.matmul(pso[n], tW2[:, i * P:(i + 1) * P],
                                     uq[:, c0:c0 + cw],
                                     start=False, stop=(i == HT - 1))

            # ---- epilogue: add b2_eff on ScalarE, store (fp16, both rings) ----
            for n, (c0, cw) in enumerate(CHUNKS):
                ob = wpool.tile([P, cw], f16, tag=f"ob{n}")
                nc.scalar.activation(ob, pso[n], Act.Identity,
                                     bias=tB2E[:, 0:1], scale=1.0)
                eng = nc.sync if n % 2 == 0 else nc.scalar
                eng.dma_start(out=dOUT.ap()[:, c0:c0 + cw], in_=ob)

    if not nc.is_finalized():
        nc.finalize()
    return nc


def _to_bf16(a):
    import ml_dtypes
    return np.asarray(a, np.float32).astype(ml_dtypes.bfloat16)


def _prep_inputs(x, embed_w, W1, b1, W2, b2, beta, thr):
    E = embed_w.astype(np.float64)
    W1d = W1.astype(np.float64)
    A1T = np.ascontiguousarray(E @ W1d[:, :H].T).astype(np.float32)   # [97, 512]
    A2T = np.ascontiguousarray(E @ W1d[:, H:].T).astype(np.float32)

    def hi_lo_pack(A):
        # [97, HT*256]: per h-tile block of (hi 128 | lo 128) columns
        hi = _to_bf16(A).reshape(P, HT, 128)
        lo = _to_bf16(A - hi.astype(np.float32).reshape(P, H)).reshape(P, HT, 128)
        return np.ascontiguousarray(
            np.concatenate([hi, lo], axis=2).reshape(P, 2 * H))

    A1P = hi_lo_pack(A1T)   # [97, 1024] bf16
    A2P = hi_lo_pack(A2T)

    pid = np.arange(NPAD)
    v0 = (pid // P).astype(np.float32)
    v1 = (pid % P).astype(np.float32)

    kq = -1.0 / ((1.0 - beta) * thr)
    qbias = (beta * (1.0 + kq * b1.astype(np.float64)))

    G = (1.0 - beta ** NSTEPS) / (1.0 - beta)
    b2e = (G * b2.astype(np.float64)
           - (beta ** 16) * W2.astype(np.float64).sum(axis=1))
    b2e_col = np.zeros((128, 1), np.float32)
    b2e_col[:P, 0] = b2e.astype(np.float32)

    # [128, 4*97] fp16: h-tile i's NEGATED W2 block in cols [i*97, (i+1)*97)
    W2TP = np.ascontiguousarray(
        (-W2.T).astype(np.float16).reshape(HT, 128, P).transpose(1, 0, 2)
        .reshape(128, HT * P))

    QBB = np.ascontiguousarray(np.concatenate([
        qbias.astype(np.float32).reshape(HT, 128).T,
        (-(beta ** 15) * qbias).astype(np.float32).reshape(HT, 128).T,
        b2e_col,
    ], axis=1))

    in_maps = []
    for k in range(NCORES):
        sl = slice(k * NLOC, (k + 1) * NLOC)
        VR = _to_bf16(np.concatenate([v0[sl], v1[sl]]).reshape(1, 2 * NLOC))
        in_maps.append({
            "VR": np.ascontiguousarray(VR),
            "A1P": A1P, "A2P": A2P, "W2TP": W2TP, "QBB": QBB,
        })
    return in_maps


def kernel(x, embed_w, W1, b1, W2, b2, beta1, beta2, thr1, thr2, **_):
    from concourse.bass_utils import run_bass_kernel_spmd

    beta = float(np.clip(np.float32(beta1), 0.1, 0.9))
    beta2c = float(np.clip(np.float32(beta2), 0.1, 0.9))
    thr = float(max(np.float32(thr1), 0.1))
    assert abs(beta - beta2c) < 1e-12, "kernel assumes beta1 == beta2"

    key = (round(beta, 9), round(thr, 9))
    if key not in _CACHE:
        _CACHE[key] = _build_bass(beta, thr)
    nc = _CACHE[key]

    in_maps = _prep_inputs(x, embed_w, W1, b1, W2, b2, beta, thr)
    res = run_bass_kernel_spmd(nc, in_maps, core_ids=list(range(NCORES)))
    T = np.concatenate([r["out"].astype(np.float32) for r in res.results],
                       axis=1)[:, :NGRID]

    pid = x[:, 0].astype(np.int64) * P + x[:, 1].astype(np.int64)
    return np.ascontiguousarray(T.T[pid]).astype(np.float32)
